# revision 5
# baseline (speedup 1.0000x reference)
"""Trainium2 Bass kernel for the pairwise-MLP GNN message-passing problem.

dro[b,n,m] = W3 . relu(W2^T relu(PhiA[b,n] @ W1a + PhiB[b,m] @ W1b + b1) + b2) + b3

Shapes (hardcoded): B=2, N=1024, M=256, D=576 (padded to 640), H1=512, H2=256.
Sharding: 8 cores over (B, N): core c handles b = c//4, n in [256*(c%4), 256*(c%4)+256).
Weights replicated. Each core computes its (256, 256) tile of dro independently.

Matmuls run in float32r (TF32-like, 11-bit mantissa, full fp32 accumulate in
PSUM) which streams at 1 cycle/row on the PE when the moving free dim >= 256.
Elementwise math is fp32 on DVE/ACT; fp32r rounding happens on op outputs that
feed the PE (required by the BIR verifier).
"""

import os
import numpy as np

B, N, M = 2, 1024, 256
D, D_PAD = 576, 640
H1, H2 = 512, 256
N_CORES = 8
N_LOC = N // 4          # 256 rows of dro per core
KT1 = D_PAD // 128      # 5 contraction tiles for layer 1
KT2 = H1 // 128         # 4 contraction tiles for layer 2
H1T = H1 // 128         # 4 partition tiles of h1
H2T = H2 // 128         # 2 partition tiles of h2
CHUNK_N = 2             # robot points per inner chunk (512 pairs)
N_CHUNKS = N_LOC // CHUNK_N
STAGE_CHUNKS = 8        # chunks per output staging buffer / DMA

_RUNNER = None


def _round_fp32r(a):
    b = np.ascontiguousarray(a, dtype=np.float32).view(np.uint32)
    return ((b + np.uint32(0x800)) & np.uint32(0xFFFFF000)).view(np.float32)


def _split_multiwaits(bir_json):
    """This container's walrus accepts only one sync-wait command per
    instruction; hoist all but the last wait onto preceding same-engine
    EventSemaphore instructions (semantically identical: consecutive waits)."""
    import orjson

    d = orjson.loads(bir_json)
    for fn in d.get("functions", []):
        for blk in fn.get("blocks", []):
            insts = blk.get("instructions") or []
            out = []
            for inst in insts:
                si = inst.get("sync_info")
                waits = (si or {}).get("on_wait") or []
                if len(waits) > 1:
                    for j, w in enumerate(waits[:-1]):
                        out.append({
                            "debug": inst.get("debug", 0),
                            "engine": inst["engine"],
                            "ins": [],
                            "name": f"{inst['name']}-mw{j}",
                            "opcode": "EventSemaphore",
                            "outs": [],
                            "sync_info": {"on_update": [], "on_wait": [w]},
                        })
                    si["on_wait"] = [waits[-1]]
                out.append(inst)
            blk["instructions"] = out
    return orjson.dumps(d)


def _install_birfix():
    import concourse.bass2jax as b2j

    if getattr(b2j, "_multiwait_patched", False):
        return
    orig = b2j.compile_bir_kernel

    def patched(bir_json, tmpdir, neff_name="file.neff"):
        return orig(_split_multiwaits(bir_json), tmpdir, neff_name=neff_name)

    b2j.compile_bir_kernel = patched
    b2j._multiwait_patched = True


def _build_nc():
    import concourse.bass as bass
    import concourse.tile as tile
    import concourse.mybir as mybir

    f32 = mybir.dt.float32
    f32r = mybir.dt.float32r
    add_op = mybir.AluOpType.add
    max_op = mybir.AluOpType.max

    nc = bass.Bass("TRN2", target_bir_lowering=False, debug=False,
                   num_devices=N_CORES)

    pa_ext = nc.dram_tensor("pa_t", [D_PAD, N_LOC], f32r, kind="ExternalInput")
    pb_ext = nc.dram_tensor("pb_t", [D_PAD, M], f32r, kind="ExternalInput")
    w1a_ext = nc.dram_tensor("w1a", [D_PAD, H1], f32r, kind="ExternalInput")
    w1b_ext = nc.dram_tensor("w1b", [D_PAD, H1], f32r, kind="ExternalInput")
    w2_ext = nc.dram_tensor("w2", [H1, H2], f32r, kind="ExternalInput")
    w3_ext = nc.dram_tensor("w3", [H2, 1], f32r, kind="ExternalInput")
    b1_ext = nc.dram_tensor("b1c", [H1, 1], f32, kind="ExternalInput")
    b2_ext = nc.dram_tensor("b2c", [H2, 1], f32, kind="ExternalInput")
    dro_ext = nc.dram_tensor("dro", [1, N_LOC * M], f32, kind="ExternalOutput")

    with tile.TileContext(nc) as tc:
        with tc.tile_pool(name="consts", bufs=1) as consts, \
             tc.tile_pool(name="proj", bufs=1) as proj:

            # ---- load constants ----
            pa_sb = []
            pb_sb = []
            w1a_sb = []
            w1b_sb = []
            for kt in range(KT1):
                t = consts.tile([128, N_LOC], f32r, tag=f"pa{kt}")
                nc.sync.dma_start(out=t, in_=pa_ext[kt * 128:(kt + 1) * 128, :])
                pa_sb.append(t)
                t = consts.tile([128, M], f32r, tag=f"pb{kt}")
                nc.sync.dma_start(out=t, in_=pb_ext[kt * 128:(kt + 1) * 128, :])
                pb_sb.append(t)
                t = consts.tile([128, H1], f32r, tag=f"w1a{kt}")
                nc.sync.dma_start(out=t, in_=w1a_ext[kt * 128:(kt + 1) * 128, :])
                w1a_sb.append(t)
                t = consts.tile([128, H1], f32r, tag=f"w1b{kt}")
                nc.sync.dma_start(out=t, in_=w1b_ext[kt * 128:(kt + 1) * 128, :])
                w1b_sb.append(t)
            w2_sb = []
            for kt in range(KT2):
                t = consts.tile([128, H2], f32r, tag=f"w2{kt}")
                nc.sync.dma_start(out=t, in_=w2_ext[kt * 128:(kt + 1) * 128, :])
                w2_sb.append(t)
            b1_sb = []
            for ht in range(H1T):
                t = consts.tile([128, 1], f32, tag=f"b1{ht}")
                nc.sync.dma_start(out=t, in_=b1_ext[ht * 128:(ht + 1) * 128, :])
                b1_sb.append(t)
            b2_sb = []
            w3_sb = []
            for ht in range(H2T):
                t = consts.tile([128, 1], f32, tag=f"b2{ht}")
                nc.sync.dma_start(out=t, in_=b2_ext[ht * 128:(ht + 1) * 128, :])
                b2_sb.append(t)
                t = consts.tile([128, 1], f32r, tag=f"w3{ht}")
                nc.sync.dma_start(out=t, in_=w3_ext[ht * 128:(ht + 1) * 128, :])
                w3_sb.append(t)

            # ---- stage A: projections a_projT (h1, n) and bplus = b_projT + b1 ----
            a_proj = []
            bplus = []
            with tc.tile_pool(name="apsum", bufs=2, space="PSUM") as apsum:
                for ht in range(H1T):
                    ps = apsum.tile([128, N_LOC], f32, tag="ps")
                    for kt in range(KT1):
                        nc.tensor.matmul(
                            ps, w1a_sb[kt][:, ht * 128:(ht + 1) * 128],
                            pa_sb[kt], start=(kt == 0), stop=(kt == KT1 - 1))
                    t = proj.tile([128, N_LOC], f32, tag=f"ap{ht}")
                    nc.scalar.copy(t, ps)
                    a_proj.append(t)
                for ht in range(H1T):
                    ps = apsum.tile([128, M], f32, tag="ps")
                    for kt in range(KT1):
                        nc.tensor.matmul(
                            ps, w1b_sb[kt][:, ht * 128:(ht + 1) * 128],
                            pb_sb[kt], start=(kt == 0), stop=(kt == KT1 - 1))
                    t = proj.tile([128, M], f32, tag=f"bp{ht}")
                    nc.scalar.activation(
                        t, ps, mybir.ActivationFunctionType.Identity,
                        bias=b1_sb[ht])
                    bplus.append(t)

            # ---- stage B: pair loop ----
            PAIRS = CHUNK_N * M  # 512, free dim of z2 matmuls
            with tc.tile_pool(name="hpool", bufs=3) as hpool, \
                 tc.tile_pool(name="rpool", bufs=3) as rpool, \
                 tc.tile_pool(name="spool", bufs=2) as spool, \
                 tc.tile_pool(name="zpsum", bufs=2, space="PSUM") as zpsum, \
                 tc.tile_pool(name="dpsum", bufs=2, space="PSUM") as dpsum:
                stg = None
                for c in range(N_CHUNKS):
                    # h1T tiles for this chunk: relu(bplus + a_col) -> fp32r
                    h1 = []
                    for kt in range(KT2):
                        t = hpool.tile([128, PAIRS], f32r, tag=f"h1_{kt}")
                        for half in range(CHUNK_N):
                            n = CHUNK_N * c + half
                            nc.vector.tensor_scalar(
                                out=t[:, half * M:(half + 1) * M],
                                in0=bplus[kt],
                                scalar1=a_proj[kt][:, n:n + 1],
                                scalar2=0.0,
                                op0=add_op, op1=max_op)
                        h1.append(t)
                    # z2 = W2^T h1  (h2 on partitions), relu2 = relu(z2 + b2) -> fp32r
                    relu2 = []
                    for ht in range(H2T):
                        zps = zpsum.tile([128, PAIRS], f32, tag=f"z{ht}")
                        for kt in range(KT2):
                            nc.tensor.matmul(
                                zps, w2_sb[kt][:, ht * 128:(ht + 1) * 128],
                                h1[kt], start=(kt == 0), stop=(kt == KT2 - 1))
                        r = rpool.tile([128, PAIRS], f32r, tag=f"r{ht}")
                        nc.scalar.activation(
                            r, zps, mybir.ActivationFunctionType.Relu,
                            bias=b2_sb[ht])
                        relu2.append(r)
                    # dro row-pair = w3 . relu2  (contraction over h2 partitions)
                    dps = dpsum.tile([1, PAIRS], f32, tag="d")
                    for ht in range(H2T):
                        nc.tensor.matmul(dps, w3_sb[ht], relu2[ht],
                                         start=(ht == 0), stop=(ht == H2T - 1))
                    # stage the row pair; DMA out every STAGE_CHUNKS chunks
                    s = c % STAGE_CHUNKS
                    if s == 0:
                        stg = spool.tile([1, STAGE_CHUNKS * PAIRS], f32, tag="stg")
                    nc.scalar.copy(stg[:, s * PAIRS:(s + 1) * PAIRS], dps)
                    if s == STAGE_CHUNKS - 1:
                        g = c // STAGE_CHUNKS
                        sz = STAGE_CHUNKS * PAIRS
                        nc.sync.dma_start(
                            out=dro_ext[:, g * sz:(g + 1) * sz], in_=stg)
    return nc


class _Runner:
    def __init__(self):
        _install_birfix()
        import jax
        import numpy as _np
        from jax.sharding import Mesh, PartitionSpec
        from jax.experimental.shard_map import shard_map
        import concourse.bass2jax as b2j
        import concourse.mybir as mybir

        nc = _build_nc()
        self.nc = nc
        b2j.install_neuronx_cc_hook()

        partition_name = (nc.partition_id_tensor.name
                          if nc.partition_id_tensor else None)
        in_names, out_names, out_avals, zero_outs = [], [], [], []
        for alloc in nc.m.functions[0].allocations:
            if not isinstance(alloc, mybir.MemoryLocationSet):
                continue
            name = alloc.memorylocations[0].name
            if alloc.kind == "ExternalInput":
                if name != partition_name:
                    in_names.append(name)
            elif alloc.kind == "ExternalOutput":
                shape = tuple(alloc.tensor_shape)
                dtype = mybir.dt.np(alloc.dtype)
                out_names.append(name)
                out_avals.append(jax.core.ShapedArray(shape, dtype))
                zero_outs.append(_np.zeros(shape, dtype))
        n_params = len(in_names)
        self.in_names = list(in_names)
        self.out_names = out_names
        self.zero_outs = zero_outs
        bind_names = list(in_names) + list(out_names)
        if partition_name is not None:
            bind_names.append(partition_name)

        def _body(*args):
            operands = list(args)
            if partition_name is not None:
                operands.append(b2j.partition_id_tensor())
            outs = b2j._bass_exec_p.bind(
                *operands,
                out_avals=tuple(out_avals),
                in_names=tuple(bind_names),
                out_names=tuple(out_names),
                lowering_input_output_aliases=(),
                sim_require_finite=True,
                sim_require_nnan=True,
                nc=nc,
            )
            return tuple(outs)

        devices = jax.devices()[:N_CORES]
        assert len(devices) == N_CORES, f"need {N_CORES} cores, have {devices}"
        mesh = Mesh(_np.asarray(devices), ("core",))
        n_outs = len(out_names)
        self.fn = jax.jit(
            shard_map(_body, mesh=mesh,
                      in_specs=(PartitionSpec("core"),) * (n_params + n_outs),
                      out_specs=(PartitionSpec("core"),) * n_outs,
                      check_rep=False),
            keep_unused=True,
        )
        self.jax = jax

    def run(self, per_core_maps):
        np_ = np
        concat_in = [
            np_.concatenate([m[name] for m in per_core_maps], axis=0)
            for name in self.in_names
        ]
        concat_zero = [
            np_.zeros((N_CORES * z.shape[0], *z.shape[1:]), z.dtype)
            for z in self.zero_outs
        ]
        out = self.fn(*concat_in, *concat_zero)
        out = [np_.asarray(o) for o in out]
        return out

    def time_ns(self, per_core_maps, iters=10):
        """Best-effort device execution time: device-resident inputs,
        min wall time over iters."""
        import time
        jax = self.jax
        concat_in = [
            np.concatenate([m[name] for m in per_core_maps], axis=0)
            for name in self.in_names
        ]
        concat_zero = [
            np.zeros((N_CORES * z.shape[0], *z.shape[1:]), z.dtype)
            for z in self.zero_outs
        ]
        dev_in = [jax.device_put(a) for a in concat_in]
        dev_zero = [jax.device_put(a) for a in concat_zero]
        # warmup
        r = self.fn(*dev_in, *dev_zero)
        jax.block_until_ready(r)
        best = float("inf")
        for _ in range(iters):
            t0 = time.perf_counter_ns()
            r = self.fn(*dev_in, *dev_zero)
            jax.block_until_ready(r)
            dt = time.perf_counter_ns() - t0
            best = min(best, dt)
        return best


def _get_runner():
    global _RUNNER
    if _RUNNER is None:
        _RUNNER = _Runner()
    return _RUNNER


def build_per_core(inputs):
    """Shard + lay out the full inputs into per-core input maps."""
    Phi_A = np.asarray(inputs["Phi_A"], dtype=np.float32)
    Phi_B = np.asarray(inputs["Phi_B"], dtype=np.float32)
    W1a = np.asarray(inputs["W1a"], dtype=np.float32)
    W1b = np.asarray(inputs["W1b"], dtype=np.float32)
    W2 = np.asarray(inputs["W2"], dtype=np.float32)
    W3 = np.asarray(inputs["W3"], dtype=np.float32)
    b1 = np.asarray(inputs["b1"], dtype=np.float32)
    b2 = np.asarray(inputs["b2"], dtype=np.float32)

    # pad contraction dim D -> 640 with zeros, pre-round fp32r operands
    w1a_p = np.zeros((D_PAD, H1), np.float32)
    w1a_p[:D] = W1a
    w1b_p = np.zeros((D_PAD, H1), np.float32)
    w1b_p[:D] = W1b
    w1a_p = _round_fp32r(w1a_p)
    w1b_p = _round_fp32r(w1b_p)
    w2_r = _round_fp32r(W2)
    w3_r = _round_fp32r(W3.reshape(H2, 1))
    b1c = b1.reshape(H1, 1)
    b2c = b2.reshape(H2, 1)

    per_core = []
    for c in range(N_CORES):
        b = c // 4
        n0 = (c % 4) * N_LOC
        pa = np.zeros((D_PAD, N_LOC), np.float32)
        pa[:D] = Phi_A[b, n0:n0 + N_LOC, :].T
        pb = np.zeros((D_PAD, M), np.float32)
        pb[:D] = Phi_B[b].T
        per_core.append({
            "pa_t": _round_fp32r(pa),
            "pb_t": _round_fp32r(pb),
            "w1a": w1a_p,
            "w1b": w1b_p,
            "w2": w2_r,
            "w3": w3_r,
            "b1c": b1c,
            "b2c": b2c,
        })
    return per_core


def kernel(Phi_A, Phi_B, W1a, W1b, b1, W2, b2, W3, b3):
    b3 = np.asarray(b3, dtype=np.float32)
    per_core = build_per_core({
        "Phi_A": Phi_A, "Phi_B": Phi_B, "W1a": W1a, "W1b": W1b,
        "b1": b1, "W2": W2, "b2": b2, "W3": W3,
    })
    runner = _get_runner()
    outs = runner.run(per_core)
    dro_flat = outs[runner.out_names.index("dro")]  # (N_CORES, N_LOC*M)
    dro = np.empty((B, N, M), np.float32)
    for c in range(N_CORES):
        b = c // 4
        n0 = (c % 4) * N_LOC
        dro[b, n0:n0 + N_LOC, :] = dro_flat[c].reshape(N_LOC, M)
    return dro + b3.reshape(-1)[0]


# revision 10
# speedup vs baseline: 250.9731x; 250.9731x over previous
"""Trainium2 Bass kernel for the pairwise-MLP GNN message-passing problem.

dro[b,n,m] = W3 . relu(W2^T relu(PhiA[b,n] @ W1a + PhiB[b,m] @ W1b + b1) + b2) + b3

Shapes (hardcoded): B=2, N=1024, M=256, D=576 (padded to 640), H1=512, H2=256.
Sharding: 8 cores over (B, N): core c handles b = c//4, n in [256*(c%4), 256*(c%4)+256).
Weights replicated. Each core computes its (256, 256) tile of dro independently.

Matmuls run in float32r (TF32-like, 11-bit mantissa, full fp32 accumulate in
PSUM) which streams at 1 cycle/row on the PE when the moving free dim >= 256.
Elementwise math is fp32 on DVE/ACT; fp32r rounding happens on op outputs that
feed the PE (required by the BIR verifier).
"""

import os
import numpy as np

B, N, M = 2, 1024, 256
D, D_PAD = 576, 640
H1, H2 = 512, 256
N_CORES = 8
N_LOC = N // 4          # 256 rows of dro per core
KT1 = D_PAD // 128      # 5 contraction tiles for layer 1
KT2 = H1 // 128         # 4 contraction tiles for layer 2
H1T = H1 // 128         # 4 partition tiles of h1
H2T = H2 // 128         # 2 partition tiles of h2
CHUNK_N = 2             # robot points per inner chunk (512 pairs)
N_CHUNKS = N_LOC // CHUNK_N
STAGE_CHUNKS = 8        # chunks per output staging buffer / DMA

_RUNNER = None


def _round_fp32r(a):
    b = np.ascontiguousarray(a, dtype=np.float32).view(np.uint32)
    return ((b + np.uint32(0x800)) & np.uint32(0xFFFFF000)).view(np.float32)


def _split_multiwaits(bir_json):
    """This container's walrus accepts only one sync-wait command per
    instruction; hoist all but the last wait onto preceding same-engine
    EventSemaphore instructions (semantically identical: consecutive waits)."""
    import orjson

    d = orjson.loads(bir_json)
    for fn in d.get("functions", []):
        for blk in fn.get("blocks", []):
            insts = blk.get("instructions") or []
            out = []
            for inst in insts:
                si = inst.get("sync_info")
                waits = (si or {}).get("on_wait") or []
                if len(waits) > 1:
                    for j, w in enumerate(waits[:-1]):
                        out.append({
                            "debug": inst.get("debug", 0),
                            "engine": inst["engine"],
                            "ins": [],
                            "name": f"{inst['name']}-mw{j}",
                            "opcode": "EventSemaphore",
                            "outs": [],
                            "sync_info": {"on_update": [], "on_wait": [w]},
                        })
                    si["on_wait"] = [waits[-1]]
                out.append(inst)
            blk["instructions"] = out
    return orjson.dumps(d)


def _install_birfix():
    import concourse.bass2jax as b2j

    if getattr(b2j, "_multiwait_patched", False):
        return
    orig = b2j.compile_bir_kernel

    def patched(bir_json, tmpdir, neff_name="file.neff"):
        return orig(_split_multiwaits(bir_json), tmpdir, neff_name=neff_name)

    b2j.compile_bir_kernel = patched
    b2j._multiwait_patched = True


def _build_nc(repeat=1):
    import concourse.bass as bass
    import concourse.tile as tile
    import concourse.mybir as mybir

    f32 = mybir.dt.float32
    f32r = mybir.dt.float32r
    add_op = mybir.AluOpType.add
    max_op = mybir.AluOpType.max

    nc = bass.Bass("TRN2", target_bir_lowering=False, debug=False,
                   num_devices=N_CORES)

    pa_ext = nc.dram_tensor("pa_t", [D_PAD, N_LOC], f32r, kind="ExternalInput")
    pb_ext = nc.dram_tensor("pb_t", [D_PAD, M], f32r, kind="ExternalInput")
    w1a_ext = nc.dram_tensor("w1a", [D_PAD, H1], f32r, kind="ExternalInput")
    w1b_ext = nc.dram_tensor("w1b", [D_PAD, H1], f32r, kind="ExternalInput")
    w2_ext = nc.dram_tensor("w2", [H1, H2], f32r, kind="ExternalInput")
    w3_ext = nc.dram_tensor("w3", [H2, 1], f32r, kind="ExternalInput")
    b1_ext = nc.dram_tensor("b1c", [H1, 1], f32, kind="ExternalInput")
    b2_ext = nc.dram_tensor("b2c", [H2, 1], f32, kind="ExternalInput")
    dro_ext = nc.dram_tensor("dro", [1, N_LOC * M], f32, kind="ExternalOutput")

    with tile.TileContext(nc) as tc:
        with tc.tile_pool(name="consts", bufs=1) as consts, \
             tc.tile_pool(name="proj", bufs=1) as proj:

            # ---- load constants ----
            pa_sb = []
            pb_sb = []
            w1a_sb = []
            w1b_sb = []
            for kt in range(KT1):
                t = consts.tile([128, N_LOC], f32r, tag=f"pa{kt}")
                nc.sync.dma_start(out=t, in_=pa_ext[kt * 128:(kt + 1) * 128, :])
                pa_sb.append(t)
                t = consts.tile([128, M], f32r, tag=f"pb{kt}")
                nc.sync.dma_start(out=t, in_=pb_ext[kt * 128:(kt + 1) * 128, :])
                pb_sb.append(t)
                t = consts.tile([128, H1], f32r, tag=f"w1a{kt}")
                nc.sync.dma_start(out=t, in_=w1a_ext[kt * 128:(kt + 1) * 128, :])
                w1a_sb.append(t)
                t = consts.tile([128, H1], f32r, tag=f"w1b{kt}")
                nc.sync.dma_start(out=t, in_=w1b_ext[kt * 128:(kt + 1) * 128, :])
                w1b_sb.append(t)
            w2_sb = []
            for kt in range(KT2):
                t = consts.tile([128, H2], f32r, tag=f"w2{kt}")
                nc.sync.dma_start(out=t, in_=w2_ext[kt * 128:(kt + 1) * 128, :])
                w2_sb.append(t)
            b1_sb = []
            for ht in range(H1T):
                t = consts.tile([128, 1], f32, tag=f"b1{ht}")
                nc.sync.dma_start(out=t, in_=b1_ext[ht * 128:(ht + 1) * 128, :])
                b1_sb.append(t)
            b2_sb = []
            w3_sb = []
            for ht in range(H2T):
                t = consts.tile([128, 1], f32, tag=f"b2{ht}")
                nc.sync.dma_start(out=t, in_=b2_ext[ht * 128:(ht + 1) * 128, :])
                b2_sb.append(t)
                t = consts.tile([128, 1], f32r, tag=f"w3{ht}")
                nc.sync.dma_start(out=t, in_=w3_ext[ht * 128:(ht + 1) * 128, :])
                w3_sb.append(t)

            # ---- stage A: projections a_projT (h1, n) and bplus = b_projT + b1 ----
            a_proj = []
            bplus = []
            with tc.tile_pool(name="apsum", bufs=2, space="PSUM") as apsum:
                for ht in range(H1T):
                    ps = apsum.tile([128, N_LOC], f32, tag="ps")
                    for kt in range(KT1):
                        nc.tensor.matmul(
                            ps, w1a_sb[kt][:, ht * 128:(ht + 1) * 128],
                            pa_sb[kt], start=(kt == 0), stop=(kt == KT1 - 1))
                    t = proj.tile([128, N_LOC], f32, tag=f"ap{ht}")
                    nc.scalar.copy(t, ps)
                    a_proj.append(t)
                for ht in range(H1T):
                    ps = apsum.tile([128, M], f32, tag="ps")
                    for kt in range(KT1):
                        nc.tensor.matmul(
                            ps, w1b_sb[kt][:, ht * 128:(ht + 1) * 128],
                            pb_sb[kt], start=(kt == 0), stop=(kt == KT1 - 1))
                    t = proj.tile([128, M], f32, tag=f"bp{ht}")
                    nc.scalar.activation(
                        t, ps, mybir.ActivationFunctionType.Identity,
                        bias=b1_sb[ht])
                    bplus.append(t)

            # ---- stage B: pair loop ----
            PAIRS = CHUNK_N * M  # 512, free dim of z2 matmuls
            with tc.tile_pool(name="hpool", bufs=4) as hpool, \
                 tc.tile_pool(name="rpool", bufs=4) as rpool, \
                 tc.tile_pool(name="spool", bufs=3) as spool, \
                 tc.tile_pool(name="zpsum", bufs=3, space="PSUM") as zpsum, \
                 tc.tile_pool(name="dpsum", bufs=2, space="PSUM") as dpsum:
                stg = None
                for c_rep in range(repeat * N_CHUNKS):
                    c = c_rep % N_CHUNKS
                    # h1T tiles for this chunk: relu(bplus + a_col) -> fp32r
                    h1 = []
                    for kt in range(KT2):
                        t = hpool.tile([128, PAIRS], f32r, tag=f"h1_{kt}")
                        for half in range(CHUNK_N):
                            n = CHUNK_N * c + half
                            nc.vector.tensor_scalar(
                                out=t[:, half * M:(half + 1) * M],
                                in0=bplus[kt],
                                scalar1=a_proj[kt][:, n:n + 1],
                                scalar2=0.0,
                                op0=add_op, op1=max_op)
                        h1.append(t)
                    # z2 = W2^T h1  (h2 on partitions), relu2 = relu(z2 + b2) -> fp32r
                    relu2 = []
                    for ht in range(H2T):
                        zps = zpsum.tile([128, PAIRS], f32, tag=f"z{ht}")
                        for kt in range(KT2):
                            nc.tensor.matmul(
                                zps, w2_sb[kt][:, ht * 128:(ht + 1) * 128],
                                h1[kt], start=(kt == 0), stop=(kt == KT2 - 1))
                        r = rpool.tile([128, PAIRS], f32r, tag=f"r{ht}")
                        nc.scalar.activation(
                            r, zps, mybir.ActivationFunctionType.Relu,
                            bias=b2_sb[ht])
                        relu2.append(r)
                    # dro row-pair = w3 . relu2  (contraction over h2 partitions)
                    dps = dpsum.tile([1, PAIRS], f32, tag="d")
                    for ht in range(H2T):
                        nc.tensor.matmul(dps, w3_sb[ht], relu2[ht],
                                         start=(ht == 0), stop=(ht == H2T - 1))
                    # stage the row pair; DMA out every STAGE_CHUNKS chunks
                    s = c % STAGE_CHUNKS
                    if s == 0:
                        stg = spool.tile([1, STAGE_CHUNKS * PAIRS], f32, tag="stg")
                    nc.scalar.copy(stg[:, s * PAIRS:(s + 1) * PAIRS], dps)
                    if s == STAGE_CHUNKS - 1:
                        g = c // STAGE_CHUNKS
                        sz = STAGE_CHUNKS * PAIRS
                        nc.sync.dma_start(
                            out=dro_ext[:, g * sz:(g + 1) * sz], in_=stg)
    return nc


class _Runner:
    def __init__(self, repeat=1):
        _install_birfix()
        import jax
        import numpy as _np
        from jax.sharding import Mesh, PartitionSpec
        from jax.experimental.shard_map import shard_map
        import concourse.bass2jax as b2j
        import concourse.mybir as mybir

        nc = _build_nc(repeat=repeat)
        self.nc = nc
        b2j.install_neuronx_cc_hook()

        partition_name = (nc.partition_id_tensor.name
                          if nc.partition_id_tensor else None)
        in_names, out_names, out_avals, zero_outs = [], [], [], []
        for alloc in nc.m.functions[0].allocations:
            if not isinstance(alloc, mybir.MemoryLocationSet):
                continue
            name = alloc.memorylocations[0].name
            if alloc.kind == "ExternalInput":
                if name != partition_name:
                    in_names.append(name)
            elif alloc.kind == "ExternalOutput":
                shape = tuple(alloc.tensor_shape)
                dtype = mybir.dt.np(alloc.dtype)
                out_names.append(name)
                out_avals.append(jax.core.ShapedArray(shape, dtype))
                zero_outs.append(_np.zeros(shape, dtype))
        n_params = len(in_names)
        self.in_names = list(in_names)
        self.out_names = out_names
        self.zero_outs = zero_outs
        bind_names = list(in_names) + list(out_names)
        if partition_name is not None:
            bind_names.append(partition_name)

        def _body(*args):
            operands = list(args)
            if partition_name is not None:
                operands.append(b2j.partition_id_tensor())
            outs = b2j._bass_exec_p.bind(
                *operands,
                out_avals=tuple(out_avals),
                in_names=tuple(bind_names),
                out_names=tuple(out_names),
                lowering_input_output_aliases=(),
                sim_require_finite=True,
                sim_require_nnan=True,
                nc=nc,
            )
            return tuple(outs)

        devices = jax.devices()[:N_CORES]
        assert len(devices) == N_CORES, f"need {N_CORES} cores, have {devices}"
        mesh = Mesh(_np.asarray(devices), ("core",))
        n_outs = len(out_names)
        self.fn = jax.jit(
            shard_map(_body, mesh=mesh,
                      in_specs=(PartitionSpec("core"),) * (n_params + n_outs),
                      out_specs=(PartitionSpec("core"),) * n_outs,
                      check_rep=False),
            keep_unused=True,
        )
        self.jax = jax

    def run(self, per_core_maps):
        np_ = np
        concat_in = [
            np_.concatenate([m[name] for m in per_core_maps], axis=0)
            for name in self.in_names
        ]
        concat_zero = [
            np_.zeros((N_CORES * z.shape[0], *z.shape[1:]), z.dtype)
            for z in self.zero_outs
        ]
        out = self.fn(*concat_in, *concat_zero)
        out = [np_.asarray(o) for o in out]
        return out

    def time_ns(self, per_core_maps, iters=10):
        """Best-effort device execution time: device-resident inputs,
        min wall time over iters."""
        import time
        jax = self.jax
        concat_in = [
            np.concatenate([m[name] for m in per_core_maps], axis=0)
            for name in self.in_names
        ]
        concat_zero = [
            np.zeros((N_CORES * z.shape[0], *z.shape[1:]), z.dtype)
            for z in self.zero_outs
        ]
        dev_in = [jax.device_put(a) for a in concat_in]
        dev_zero = [jax.device_put(a) for a in concat_zero]
        # warmup
        r = self.fn(*dev_in, *dev_zero)
        jax.block_until_ready(r)
        best = float("inf")
        for _ in range(iters):
            t0 = time.perf_counter_ns()
            r = self.fn(*dev_in, *dev_zero)
            jax.block_until_ready(r)
            dt = time.perf_counter_ns() - t0
            best = min(best, dt)
        return best


def _get_runner():
    global _RUNNER
    if _RUNNER is None:
        _RUNNER = _Runner()
    return _RUNNER


def build_per_core(inputs):
    """Shard + lay out the full inputs into per-core input maps."""
    Phi_A = np.asarray(inputs["Phi_A"], dtype=np.float32)
    Phi_B = np.asarray(inputs["Phi_B"], dtype=np.float32)
    W1a = np.asarray(inputs["W1a"], dtype=np.float32)
    W1b = np.asarray(inputs["W1b"], dtype=np.float32)
    W2 = np.asarray(inputs["W2"], dtype=np.float32)
    W3 = np.asarray(inputs["W3"], dtype=np.float32)
    b1 = np.asarray(inputs["b1"], dtype=np.float32)
    b2 = np.asarray(inputs["b2"], dtype=np.float32)

    # pad contraction dim D -> 640 with zeros, pre-round fp32r operands
    w1a_p = np.zeros((D_PAD, H1), np.float32)
    w1a_p[:D] = W1a
    w1b_p = np.zeros((D_PAD, H1), np.float32)
    w1b_p[:D] = W1b
    w1a_p = _round_fp32r(w1a_p)
    w1b_p = _round_fp32r(w1b_p)
    w2_r = _round_fp32r(W2)
    w3_r = _round_fp32r(W3.reshape(H2, 1))
    b1c = b1.reshape(H1, 1)
    b2c = b2.reshape(H2, 1)

    per_core = []
    for c in range(N_CORES):
        b = c // 4
        n0 = (c % 4) * N_LOC
        pa = np.zeros((D_PAD, N_LOC), np.float32)
        pa[:D] = Phi_A[b, n0:n0 + N_LOC, :].T
        pb = np.zeros((D_PAD, M), np.float32)
        pb[:D] = Phi_B[b].T
        per_core.append({
            "pa_t": _round_fp32r(pa),
            "pb_t": _round_fp32r(pb),
            "w1a": w1a_p,
            "w1b": w1b_p,
            "w2": w2_r,
            "w3": w3_r,
            "b1c": b1c,
            "b2c": b2c,
        })
    return per_core


def kernel(Phi_A, Phi_B, W1a, W1b, b1, W2, b2, W3, b3):
    b3 = np.asarray(b3, dtype=np.float32)
    per_core = build_per_core({
        "Phi_A": Phi_A, "Phi_B": Phi_B, "W1a": W1a, "W1b": W1b,
        "b1": b1, "W2": W2, "b2": b2, "W3": W3,
    })
    runner = _get_runner()
    outs = runner.run(per_core)
    dro_flat = outs[runner.out_names.index("dro")]  # (N_CORES, N_LOC*M)
    dro = np.empty((B, N, M), np.float32)
    for c in range(N_CORES):
        b = c // 4
        n0 = (c % 4) * N_LOC
        dro[b, n0:n0 + N_LOC, :] = dro_flat[c].reshape(N_LOC, M)
    return dro + b3.reshape(-1)[0]
